# revision 22
# baseline (speedup 1.0000x reference)
"""Trainium2 Bass kernel for SAGAN-style self-attention (nn_Attention_13056700580138).

Reference computation (per batch element, with N = H*W = 4096, C = 256, CK = 32):
    f  = x @ Wf + bf            [N, CK]
    g  = x @ Wg + bg            [N, CK]
    hh = x @ Wh + bh            [N, C]
    S  = g @ f^T                [N, N]
    A  = softmax(S, axis=-1)
    o  = A @ hh                 [N, C]
    out = gamma * (o @ Wo + bo) + x

Sharding: data-parallel over batch - one batch element per NeuronCore (B = 8 = n_cores).

Per-core strategy (v2 - fp8 DoubleRow attention):
  * Output projection folded through associativity:
        (A @ hh) @ Wo + bo = A @ (x @ (Wh @ Wo)) + (bh @ Wo + bo)
    (softmax rows sum to 1). hw1 = x @ Whw + bhw computed once in fp32r, then
    split hi/lo into two fp8e4 copies (hi = fp8(hw1), lo = fp8(hw1 - hi)) so the
    value matmul runs in fp8 with ~bf16-level value precision.
  * Scores are computed transposed (S^T tiles [128 keys, 512 queries], fp32r,
    4 key blocks concurrently in tile_position row groups), two key blocks per
    2-bank PSUM tile so one ACT instruction exps 1024 columns.
  * pass-1: ACT exp -> et16 = bf16(e^s). No max subtraction needed (|s| < ~60
    fits fp32/bf16 by construction).
  * Row sums S_q = sum_k e^s ride the PE as moving-dim-1 matmuls (stationary
    [128 keys, 128 queries] et16 chunk x ones column -> [128q, 1] PSUM), which
    the cost model and hardware weight-preload make nearly free.
  * pass-2: et8 = fp8e4(et16 * (1/S_q)) - one tensor_tensor multiply per tile
    (DVE/Pool split), with 1/S_q replicated to all partitions by doubling DMAs.
    The softmax weights are then guaranteed in [0, 1]: no fp8 overflow, no
    data-dependent shift estimation.
  * Value matmul: fp8 DoubleRow - each instruction contracts two 128-key
    blocks (stationary et8 pair, moving hw1 pair) at half cycle cost; two
    chains (hw1-hi, hw1-lo) accumulate into the same PSUM.
  * Normalization sums den = sum_k et8 ride the PE the same way (DoubleRow,
    moving dim 1), so the final epilogue is one reciprocal + one fused
    multiply-add per [128, 256] block: out = gamma*(num/den) + x.
"""

from contextlib import ExitStack

import numpy as np

import bass_rust
import concourse.bass as bass
import concourse.mybir as mybir
import concourse.tile as tile
from concourse.bass_utils import run_bass_kernel_spmd
from concourse.masks import make_identity
from concourse.vector_clock import ScopedClock

FP = mybir.dt.float32
FPR = mybir.dt.float32r
BF = mybir.dt.bfloat16
F8 = mybir.dt.float8e4
AF = mybir.ActivationFunctionType
ALU = mybir.AluOpType
DR = mybir.MatmulPerfMode.DoubleRow

B, H, W, C = 8, 64, 64, 256
CK = C // 8
N = H * W  # 4096
NCORES = 8


# --- workaround: walrus in this toolchain lowers at most one sync-wait per SP
# CTRL instruction, but TileContext's final drain carries one wait per busy
# processor. Split them across single-wait carrier nops (same engine queue,
# program order => identical semantics).
def _split_drain_and_barrier(self, tick_clock, wait_clock):
    nc = self.nc
    ticks = list(eval(repr(tick_clock.global_clock).replace("VectorClock", "")))
    nproc = len(ticks)
    for i, t in enumerate(ticks):
        if t > 0:
            sub = [0] * nproc
            sub[i] = t
            carrier = nc.sync.nop(nofuse=True, hint="drain_split_wait")
            wait_clock.add_sem_waits(
                carrier.ins, ScopedClock({None: bass_rust.VectorClock(sub)})
            )
    nc.sync.drain()
    nc.all_engine_barrier()
    assert self.sems is not None
    popped = nc._tile_sem_poison_stack.pop()
    assert popped is self._sem_poison
    nc.clear_and_free_semaphores(list(self.sems.allocated().values()))
    nc.all_engine_barrier()


tile.TileContext._drain_and_barrier = _split_drain_and_barrier


def _split_instruction_waits(nc):
    """walrus in this toolchain lowers at most one sync-wait per instruction
    for several instruction templates. After Tile scheduling, move any extra
    waits onto single-wait carrier nops inserted just before the instruction
    on the same engine queue (identical blocking semantics)."""
    cnt = 0
    for fn in nc.m.functions:
        for bb in fn.blocks:
            out = []
            changed = False
            for ins in bb.instructions:
                si = ins.sync_info
                waits = list(si.on_wait) if (si is not None and si.on_wait) else []
                if len(waits) > 1:
                    changed = True
                    for wx in waits[:-1]:
                        nop = mybir.InstNoOp(name=f"wsplit-{cnt}", ins=[], outs=[])
                        cnt += 1
                        nop.engine = ins.engine
                        nop.sync_info = mybir.SyncInfo(on_wait=[wx], on_update=[])
                        nc.register_instruction(nop, overwrite=True)
                        out.append(nop)
                    si.on_wait = [waits[-1]]
                out.append(ins)
            if changed:
                bb.instructions = out
    return nc


def _emit(ctx, nc, tc, t_in, t_out):
    x_d = t_in["x"]

    singles = ctx.enter_context(tc.tile_pool(name="singles", bufs=1))
    pre_ctx = ExitStack()
    psum_pre = pre_ctx.enter_context(tc.tile_pool(name="psum_pre", bufs=4, space="PSUM"))

    # Wh/Wo first: the Whw precompute sits at the head of the PE queue and must
    # not head-block the x transposes behind a late weight DMA.
    wh_sb = singles.tile([128, 2, C], FP)
    wo_sb = singles.tile([128, 2, C], FPR)
    for kc in range(2):
        nc.sync.dma_start(out=wh_sb[:, kc, :], in_=t_in["Wh"][kc * 128:(kc + 1) * 128, :])
        nc.sync.dma_start(out=wo_sb[:, kc, :], in_=t_in["Wo"][kc * 128:(kc + 1) * 128, :].bitcast(FPR))

    # x, split into 8 chunks so the transposes can start on chunk 0
    x_view = x_d.ap().rearrange("(t p) c -> p t c", p=128)
    x_pix = []
    for q in range(8):
        xp = singles.tile([128, 4, C], FP, name=f"x_pix{q}")
        if q == 0:  # split the first chunk so the first transpose starts sooner
            nc.sync.dma_start(out=xp[:, 0:2, :], in_=x_view[:, 0:2, :])
            nc.sync.dma_start(out=xp[:, 2:4, :], in_=x_view[:, 2:4, :])
        else:
            nc.sync.dma_start(out=xp[:], in_=x_view[:, 4 * q:4 * (q + 1), :])
        x_pix.append(xp)

    identity = singles.tile([128, 128], FP)
    make_identity(nc, identity[:])

    # remaining weights / biases / gamma
    wf_sb = singles.tile([128, 2, CK], FPR)
    wg_sb = singles.tile([128, 2, CK], FPR)
    for kc in range(2):
        nc.sync.dma_start(out=wf_sb[:, kc, :], in_=t_in["Wf"][kc * 128:(kc + 1) * 128, :].bitcast(FPR))
        nc.sync.dma_start(out=wg_sb[:, kc, :], in_=t_in["Wg"][kc * 128:(kc + 1) * 128, :].bitcast(FPR))

    bf_rep = singles.tile([128, 1], FP)
    bg_rep = singles.tile([128, 1], FP)
    for t in range(4):
        nc.sync.dma_start(out=bf_rep[32 * t:32 * t + 32, :], in_=t_in["bf"][:].unsqueeze(1))
        nc.sync.dma_start(out=bg_rep[32 * t:32 * t + 32, :], in_=t_in["bg"][:].unsqueeze(1))

    bh_col = singles.tile([128, 2, 1], FPR)
    bo_row = singles.tile([1, C], FPR)
    for kc in range(2):
        nc.sync.dma_start(
            out=bh_col[:, kc, :], in_=t_in["bh"][kc * 128:(kc + 1) * 128].unsqueeze(1).bitcast(FPR)
        )
    nc.sync.dma_start(out=bo_row[0:1, :], in_=t_in["bo"][:].unsqueeze(0).bitcast(FPR))

    ones_f = singles.tile([1, 128], FP)
    nc.vector.memset(ones_f[:], 1.0)
    ones_col = singles.tile([1, 128], FPR)
    nc.vector.tensor_copy(out=ones_col[:], in_=ones_f[:])
    ones16 = singles.tile([128, 1], BF)
    nc.vector.memset(ones16[:], 1.0)
    ones8 = singles.tile([128, 2, 1], F8)
    nc.vector.memset(ones8[:], 1.0)

    gamma_rep = singles.tile([128, 1], FP)
    nc.sync.dma_start(out=gamma_rep[0:1, :], in_=t_in["gamma"][:].unsqueeze(0))
    for step in range(7):  # doubling broadcast 1 -> 128 partitions
        w_ = 1 << step
        nc.sync.dma_start(out=gamma_rep[w_:2 * w_, :], in_=gamma_rep[0:w_, :])

    # ---------------- fused output projection: Whw = Wh @ Wo, bhw = bh@Wo + bo
    whT = singles.tile([128, 2, C], FPR)  # [m % 128, m // 128, i] = Wh[i, m]
    for mc in range(2):
        for ib in range(2):
            pt = psum_pre.tile([128, 128], FP, tag="transp", bufs=3, name=f"ptw_{mc}_{ib}")
            nc.tensor.transpose(
                pt[:], wh_sb[:, ib, 128 * mc:128 * (mc + 1)], identity[:]
            )
            nc.vector.tensor_copy(out=whT[:, mc, 128 * ib:128 * (ib + 1)], in_=pt[:])

    whw_sb = singles.tile([128, 2, C], FPR)  # [i % 128, i // 128, o]
    for ib in range(2):
        ps = psum_pre.tile([128, C], FP, tag="pre", bufs=2, name=f"psw{ib}")
        for mc in range(2):
            nc.tensor.matmul(
                ps[:],
                whT[:, mc, 128 * ib:128 * (ib + 1)],
                wo_sb[:, mc, :],
                start=(mc == 0),
                stop=(mc == 1),
            )
        nc.vector.tensor_copy(out=whw_sb[:, ib, :], in_=ps[:])

    bhw_bc = singles.tile([128, C], FPR)  # (bh @ Wo + bo) broadcast to all parts
    ps_b = psum_pre.tile([1, C], FP, tag="pre", bufs=2)
    for kc in range(2):
        nc.tensor.matmul(
            ps_b[:], bh_col[:, kc, :], wo_sb[:, kc, :], start=(kc == 0), stop=False
        )
    nc.tensor.matmul(
        ps_b[:], ones_col[0:1, 0:1], bo_row[:], start=False, stop=True
    )
    nc.vector.tensor_copy(out=bhw_bc[0:1, :], in_=ps_b[:])
    for step in range(7):
        w_ = 1 << step
        nc.sync.dma_start(out=bhw_bc[w_:2 * w_, :], in_=bhw_bc[0:w_, :])

    # ---- x^T transposes pipelined with f/g projections, per 512-pixel slice
    # row group t owns keys [1024*t, 1024*(t+1)); fT4[32t:32t+32, j*128:(j+1)*128]
    # holds f^T for global key block 8t+j. gT4 replicates g^T into all 4 groups.
    xtp = pre_ctx.enter_context(tc.tile_pool(name="xtp", bufs=1))
    xT = [[xtp.tile([128, 512], FPR, name=f"xT_{kc}_{s}") for s in range(8)] for kc in range(2)]
    fT4 = singles.tile([128, 1024], FPR)
    gT4 = singles.tile([128, N], FPR)
    fT_flat = singles.tile([32, N], FPR)
    # hw1 = x @ Whw + bhw, split hi/lo fp8, pixel-major [128, t, g4, C] with
    # key block kb = 8t + g4 (attention iteration g4 uses pairs (g4, 8+g4) and
    # (16+g4, 24+g4), i.e. fixed g4, t strided - a regular stride-8C AP).
    hw1hi = singles.tile([128, 4, 8, C], F8)
    hw1lo = singles.tile([128, 4, 8, C], F8)

    for s in range(8):
        for kb in range(4 * s, 4 * s + 4):
            for kc in range(2):
                pt = psum_pre.tile([128, 128], FP, tag="transp", name=f"pt_{kb}_{kc}", bufs=3)
                nc.tensor.transpose(
                    pt[:], x_pix[kb // 4][:, kb % 4, 128 * kc:128 * (kc + 1)], identity[:]
                )
                dst = xT[kc][s][:, 128 * (kb % 4):128 * (kb % 4 + 1)]
                if kc == 0:
                    nc.vector.tensor_copy(out=dst, in_=pt[:])
                else:
                    nc.scalar.activation(out=dst, in_=pt[:], func=AF.Identity, bias=0.0)
        psf = psum_pre.tile([32, 512], FP, tag="pre", name=f"psf{s}", bufs=2)
        for kc in range(2):
            nc.tensor.matmul(
                psf[:],
                wf_sb[:, kc, :],
                xT[kc][s][:],
                start=(kc == 0),
                stop=(kc == 1),
            )
        nc.scalar.activation(
            out=fT_flat[0:32, 512 * s:512 * (s + 1)],
            in_=psf[:],
            func=AF.Identity,
            bias=bf_rep[0:32, :],
        )
        psg = psum_pre.tile([32, 512], FP, tag="pre", name=f"psg{s}", bufs=2)
        for kc in range(2):
            nc.tensor.matmul(
                psg[:],
                wg_sb[:, kc, :],
                xT[kc][s][:],
                start=(kc == 0),
                stop=(kc == 1),
            )
        nc.scalar.activation(
            out=gT4[0:32, 512 * s:512 * (s + 1)],
            in_=psg[:],
            func=AF.Identity,
            bias=bg_rep[0:32, :],
        )
        # hw1 for this slice's pixel blocks: x @ Whw + bhw -> fp8 hi/lo
        for kb in range(4 * s, 4 * s + 4):
            ps = psum_pre.tile([128, C], FP, tag="hw1", name=f"psh{kb}", bufs=2)
            for kc in range(2):
                nc.tensor.matmul(
                    ps[:],
                    xT[kc][kb // 4][:, 128 * (kb % 4):128 * (kb % 4 + 1)],
                    whw_sb[:, kc, :],
                    start=(kc == 0),
                    stop=False,
                )
            nc.tensor.matmul(
                ps[:], ones_col[0:1, :], bhw_bc[0:1, :],
                start=False, stop=True,
            )
            nc.gpsimd.tensor_copy(out=hw1hi[:, kb // 8, kb % 8, :], in_=ps[:])
            nc.vector.tensor_sub(out=hw1lo[:, kb // 8, kb % 8, :], in0=ps[:], in1=hw1hi[:, kb // 8, kb % 8, :])

    for t in range(4):
        nc.sync.dma_start(out=fT4[32 * t:32 * t + 32, :], in_=fT_flat[0:32, 1024 * t:1024 * (t + 1)])
    for t in range(1, 4):
        nc.sync.dma_start(out=gT4[32 * t:32 * t + 32, :], in_=gT4[0:32, :])

    pre_ctx.close()
    et16p = ctx.enter_context(tc.tile_pool(name="et16p", bufs=38))
    et8p = ctx.enter_context(tc.tile_pool(name="et8p", bufs=24))
    work = ctx.enter_context(tc.tile_pool(name="work", bufs=6))
    # PSUM budget (8 banks):
    #   psum_sc : 2 x [128, 1024] score tiles (4 banks)
    #   psum_o  : 2 x [128, 512]  o-accumulators, 2 query chunks per bank
    #   psum_s  : 1 x [128, 4]    row sums of et16 (one bank)
    #   psum_d  : 1 x [128, 4]    row sums of et8 / den (one bank)
    psum_sc = ctx.enter_context(tc.tile_pool(name="psum_sc", bufs=2, space="PSUM"))
    psum_o = ctx.enter_context(tc.tile_pool(name="psum_o", bufs=2, space="PSUM"))
    psum_s = ctx.enter_context(tc.tile_pool(name="psum_s", bufs=1, space="PSUM"))
    psum_d = ctx.enter_context(tc.tile_pool(name="psum_d", bufs=1, space="PSUM"))

    # ---------------- attention --------------------------------------------
    # Software pipeline over 512-query slices:
    #   step qs: scores+exp(qs) | epilogue(qs-2) | pass-2 + value matmul (qs-1)
    #            | row sums(qs) + 1/S broadcast
    # so no engine head-blocks on results produced later in the same step.
    # The last slice (qs=7) instead consumes et16 directly with bf16 value
    # matmuls (normalized by the et16 row sums), so the pipeline drain is just
    # its epilogue instead of a full exposed rescale+matmul tail.
    state = {}

    def phase_scores(qs):
        et16_tiles = []
        for g4 in range(8):
            for pair in range(2):  # pair A: t=0,1 (kb=g4, 8+g4); pair B: t=2,3
                sc = psum_sc.tile([128, 1024], FP, tag="score", name=f"sc_{qs}_{g4}_{pair}")
                for th in range(2):
                    t = 2 * pair + th
                    nc.tensor.matmul(
                        sc[:, 512 * th:512 * (th + 1)],
                        fT4[32 * t:32 * t + 32, 128 * g4:128 * (g4 + 1)],
                        gT4[32 * t:32 * t + 32, 512 * qs:512 * (qs + 1)],
                        start=True,
                        stop=True,
                        tile_position=(32 * t, 0),
                    )
                e16 = et16p.tile([128, 2, 512], BF, tag="et16", name=f"et16_{qs}_{g4}_{pair}")
                nc.scalar.activation(out=e16[:], in_=sc[:], func=AF.Exp)
                et16_tiles.append((g4, pair, e16))
        return et16_tiles

    def phase_sums(qs, et16_tiles):
        # row sums of et16 ride the PE: moving-dim-1 matmuls
        sum_ps = psum_s.tile([128, 4], FP, tag="sum", name=f"sum_{qs}")
        n = len(et16_tiles)
        for idx, (g4, pair, e16) in enumerate(et16_tiles):
            for th in range(2):
                for j in range(4):
                    nc.tensor.matmul(
                        sum_ps[:, j:j + 1],
                        e16[:, th, 128 * j:128 * (j + 1)],
                        ones16[:],
                        start=(idx == 0 and th == 0 and j == 0),
                        stop=(idx == n - 1 and th == 1 and j == 3),
                        skip_group_check=True,
                    )
        # 1/S_q -> bf16 row (gpsimd casting DMAs), then partition broadcast
        rr4 = work.tile([128, 4], FP, tag="rr4", name=f"rr4_{qs}", bufs=2)
        nc.vector.reciprocal(out=rr4[:], in_=sum_ps[:])
        rrow = work.tile([1, 512], BF, tag="rrow", name=f"rrow_{qs}", bufs=2)
        for j in range(4):
            nc.gpsimd.dma_start(out=rrow[0:1, 128 * j:128 * (j + 1)], in_=rr4[:, j:j + 1])
        rrep = work.tile([128, 2, 512], BF, tag="rrep", name=f"rrep_{qs}", bufs=2)
        for h in range(2):
            nc.gpsimd.partition_broadcast(rrep[:, h, :], rrow[0:1, :])
        return rrep

    def phase_tail(qs, et16_tiles, rrep):
        o_ps = [psum_o.tile([128, 512], FP, tag="oacc", name=f"oacc_{qs}_{c}") for c in range(2)]
        den_ps = psum_d.tile([128, 4], FP, tag="den", name=f"den_{qs}")
        for idx, (g4, pair, e16) in enumerate(et16_tiles):
            e8 = et8p.tile([128, 2, 512], F8, tag="et8", name=f"et8_{qs}_{g4}_{pair}")
            eng = nc.vector if idx % 4 < 3 else nc.gpsimd
            eng.tensor_mul(out=e8[:], in0=e16[:], in1=rrep[:])
            first = g4 == 0 and pair == 0
            last = g4 == 7 and pair == 1
            for j in range(4):
                oc = o_ps[j // 2][:, 256 * (j % 2):256 * (j % 2) + 256]
                lhs = e8[:, :, 128 * j:128 * (j + 1)]
                rhs_hi = hw1hi[:, 2 * pair:2 * pair + 2, g4, :]
                rhs_lo = hw1lo[:, 2 * pair:2 * pair + 2, g4, :]
                nc.tensor.matmul(
                    oc, lhs, rhs_hi,
                    start=(first and j % 2 == 0), stop=False,
                    perf_mode=DR, skip_group_check=True,
                )
                nc.tensor.matmul(
                    oc, lhs, rhs_lo,
                    start=False, stop=(last and j % 2 == 1),
                    perf_mode=DR, skip_group_check=True,
                )
                nc.tensor.matmul(
                    den_ps[:, j:j + 1], lhs, ones8[:],
                    start=(first and j == 0), stop=(last and j == 3),
                    perf_mode=DR, skip_group_check=True,
                )
        return o_ps, den_ps

    def phase_tail_bf16(qs, et16_tiles):
        # last slice: bf16 value matmul straight from et16 (no rescale barrier);
        # normalized by the et16 row sums.
        o_ps = [psum_o.tile([128, 512], FP, tag="oacc", name=f"oacc_{qs}_{c}") for c in range(2)]
        for idx, (g4, pair, e16) in enumerate(et16_tiles):
            first = g4 == 0 and pair == 0
            last = g4 == 7 and pair == 1
            for j in range(4):
                oc = o_ps[j // 2][:, 256 * (j % 2):256 * (j % 2) + 256]
                for th in range(2):
                    t = 2 * pair + th
                    nc.tensor.matmul(
                        oc,
                        e16[:, th, 128 * j:128 * (j + 1)],
                        hw1bf[:, t, g4, :],
                        start=(first and th == 0 and j % 2 == 0),
                        stop=(last and th == 1 and j % 2 == 1),
                        skip_group_check=True,
                    )
        return o_ps

    def phase_epilogue(qs, o_ps, den_ps):
        # out = gamma * (num / den) + x
        dinv = work.tile([128, 4], FP, tag="dinv", name=f"dinv_{qs}", bufs=2)
        nc.vector.reciprocal(out=dinv[:], in_=den_ps[:])
        nc.vector.tensor_scalar(
            out=dinv[:], in0=dinv[:], scalar1=gamma_rep[:], scalar2=None, op0=ALU.mult
        )
        for j in range(4):
            blk = 4 * qs + j
            out_sb = work.tile([128, C], FP, tag="outsb", name=f"osb_{blk}", bufs=3)
            nc.vector.scalar_tensor_tensor(
                out=out_sb[:],
                in0=o_ps[j // 2][:, 256 * (j % 2):256 * (j % 2) + 256],
                scalar=dinv[:, j:j + 1],
                in1=x_pix[blk // 4][:, blk % 4, :],
                op0=ALU.mult,
                op1=ALU.add,
            )
            nc.sync.dma_start(out=t_out[128 * blk:128 * (blk + 1), :], in_=out_sb[:])

    for step in range(11):
        if step >= 3:  # epilogue(step-3) first: frees o-banks / den for this step
            st = state.pop(step - 3)
            phase_epilogue(step - 3, st["o"], st["den"])
        if step < 8:
            state[step] = {"et16": phase_scores(step)}
        if 1 <= step <= 8:
            state[step - 1]["rrep"] = phase_sums(step - 1, state[step - 1]["et16"])
        if 2 <= step <= 9:
            st = state[step - 2]
            st["o"], st["den"] = phase_tail(step - 2, st["et16"], st["rrep"])


_CACHE = {}


def _build():
    if "nc" not in _CACHE:
        nc = bass.Bass("TRN2", target_bir_lowering=False, debug=False)
        t_in = {
            "x": nc.dram_tensor("x", [N, C], FP, kind="ExternalInput"),
            "Wf": nc.dram_tensor("Wf", [C, CK], FP, kind="ExternalInput"),
            "bf": nc.dram_tensor("bf", [CK], FP, kind="ExternalInput"),
            "Wg": nc.dram_tensor("Wg", [C, CK], FP, kind="ExternalInput"),
            "bg": nc.dram_tensor("bg", [CK], FP, kind="ExternalInput"),
            "Wh": nc.dram_tensor("Wh", [C, C], FP, kind="ExternalInput"),
            "bh": nc.dram_tensor("bh", [C], FP, kind="ExternalInput"),
            "Wo": nc.dram_tensor("Wo", [C, C], FP, kind="ExternalInput"),
            "bo": nc.dram_tensor("bo", [C], FP, kind="ExternalInput"),
            "gamma": nc.dram_tensor("gamma", [1], FP, kind="ExternalInput"),
        }
        t_out = nc.dram_tensor("out", [N, C], FP, kind="ExternalOutput")
        with tile.TileContext(nc) as tc:
            with ExitStack() as ctx:
                _emit(ctx, nc, tc, t_in, t_out)
        _split_instruction_waits(nc)
        _CACHE["nc"] = nc
    return _CACHE["nc"]


def kernel(x, Wf, bf, Wg, bg, Wh, bh, Wo, bo, gamma, _trace=False, _tmpdir=None):
    nc = _build()
    x = np.ascontiguousarray(np.asarray(x, dtype=np.float32)).reshape(B, N, C)
    w = {
        "Wf": np.ascontiguousarray(np.asarray(Wf, np.float32)),
        "bf": np.ascontiguousarray(np.asarray(bf, np.float32)),
        "Wg": np.ascontiguousarray(np.asarray(Wg, np.float32)),
        "bg": np.ascontiguousarray(np.asarray(bg, np.float32)),
        "Wh": np.ascontiguousarray(np.asarray(Wh, np.float32)),
        "bh": np.ascontiguousarray(np.asarray(bh, np.float32)),
        "Wo": np.ascontiguousarray(np.asarray(Wo, np.float32)),
        "bo": np.ascontiguousarray(np.asarray(bo, np.float32)),
        "gamma": np.ascontiguousarray(np.asarray(gamma, np.float32)),
    }
    in_maps = [dict(w, x=x[i]) for i in range(NCORES)]
    res = run_bass_kernel_spmd(
        nc, in_maps, core_ids=list(range(NCORES)), trace=_trace, tmpdir=_tmpdir
    )
    out = np.stack([res.results[i]["out"] for i in range(NCORES)])
    if _trace:
        kernel._last_result = res
    return out.reshape(B, H, W, C).astype(np.float32)


# revision 24
# speedup vs baseline: 1.0582x; 1.0582x over previous
"""Trainium2 Bass kernel for SAGAN-style self-attention (nn_Attention_13056700580138).

Reference computation (per batch element, with N = H*W = 4096, C = 256, CK = 32):
    f  = x @ Wf + bf            [N, CK]
    g  = x @ Wg + bg            [N, CK]
    hh = x @ Wh + bh            [N, C]
    S  = g @ f^T                [N, N]
    A  = softmax(S, axis=-1)
    o  = A @ hh                 [N, C]
    out = gamma * (o @ Wo + bo) + x

Sharding: data-parallel over batch - one batch element per NeuronCore (B = 8 = n_cores).

Per-core strategy (v2 - fp8 DoubleRow attention):
  * Output projection folded through associativity:
        (A @ hh) @ Wo + bo = A @ (x @ (Wh @ Wo)) + (bh @ Wo + bo)
    (softmax rows sum to 1). hw1 = x @ Whw + bhw computed once in fp32r, then
    split hi/lo into two fp8e4 copies (hi = fp8(hw1), lo = fp8(hw1 - hi)) so the
    value matmul runs in fp8 with ~bf16-level value precision.
  * Scores are computed transposed (S^T tiles [128 keys, 512 queries], fp32r,
    4 key blocks concurrently in tile_position row groups), two key blocks per
    2-bank PSUM tile so one ACT instruction exps 1024 columns.
  * pass-1: ACT exp -> et16 = bf16(e^s). No max subtraction needed (|s| < ~60
    fits fp32/bf16 by construction).
  * Row sums S_q = sum_k e^s ride the PE as moving-dim-1 matmuls (stationary
    [128 keys, 128 queries] et16 chunk x ones column -> [128q, 1] PSUM), which
    the cost model and hardware weight-preload make nearly free.
  * pass-2: et8 = fp8e4(et16 * (1/S_q)) - one tensor_tensor multiply per tile
    (DVE/Pool split), with 1/S_q replicated to all partitions by doubling DMAs.
    The softmax weights are then guaranteed in [0, 1]: no fp8 overflow, no
    data-dependent shift estimation.
  * Value matmul: fp8 DoubleRow - each instruction contracts two 128-key
    blocks (stationary et8 pair, moving hw1 pair) at half cycle cost; two
    chains (hw1-hi, hw1-lo) accumulate into the same PSUM.
  * Normalization sums den = sum_k et8 ride the PE the same way (DoubleRow,
    moving dim 1), so the final epilogue is one reciprocal + one fused
    multiply-add per [128, 256] block: out = gamma*(num/den) + x.
"""

from contextlib import ExitStack

import numpy as np

import bass_rust
import concourse.bass as bass
import concourse.mybir as mybir
import concourse.tile as tile
from concourse.bass_utils import run_bass_kernel_spmd
from concourse.masks import make_identity
from concourse.vector_clock import ScopedClock

FP = mybir.dt.float32
FPR = mybir.dt.float32r
BF = mybir.dt.bfloat16
F8 = mybir.dt.float8e4
AF = mybir.ActivationFunctionType
ALU = mybir.AluOpType
DR = mybir.MatmulPerfMode.DoubleRow

B, H, W, C = 8, 64, 64, 256
CK = C // 8
N = H * W  # 4096
NCORES = 8


# --- workaround: walrus in this toolchain lowers at most one sync-wait per SP
# CTRL instruction, but TileContext's final drain carries one wait per busy
# processor. Split them across single-wait carrier nops (same engine queue,
# program order => identical semantics).
def _split_drain_and_barrier(self, tick_clock, wait_clock):
    nc = self.nc
    ticks = list(eval(repr(tick_clock.global_clock).replace("VectorClock", "")))
    nproc = len(ticks)
    for i, t in enumerate(ticks):
        if t > 0:
            sub = [0] * nproc
            sub[i] = t
            carrier = nc.sync.nop(nofuse=True, hint="drain_split_wait")
            wait_clock.add_sem_waits(
                carrier.ins, ScopedClock({None: bass_rust.VectorClock(sub)})
            )
    nc.sync.drain()
    nc.all_engine_barrier()
    assert self.sems is not None
    popped = nc._tile_sem_poison_stack.pop()
    assert popped is self._sem_poison
    nc.clear_and_free_semaphores(list(self.sems.allocated().values()))
    nc.all_engine_barrier()


tile.TileContext._drain_and_barrier = _split_drain_and_barrier


def _split_instruction_waits(nc):
    """walrus in this toolchain lowers at most one sync-wait per instruction
    for several instruction templates. After Tile scheduling, move any extra
    waits onto single-wait carrier nops inserted just before the instruction
    on the same engine queue (identical blocking semantics)."""
    cnt = 0
    for fn in nc.m.functions:
        for bb in fn.blocks:
            out = []
            changed = False
            for ins in bb.instructions:
                si = ins.sync_info
                waits = list(si.on_wait) if (si is not None and si.on_wait) else []
                if len(waits) > 1:
                    changed = True
                    for wx in waits[:-1]:
                        nop = mybir.InstNoOp(name=f"wsplit-{cnt}", ins=[], outs=[])
                        cnt += 1
                        nop.engine = ins.engine
                        nop.sync_info = mybir.SyncInfo(on_wait=[wx], on_update=[])
                        nc.register_instruction(nop, overwrite=True)
                        out.append(nop)
                    si.on_wait = [waits[-1]]
                out.append(ins)
            if changed:
                bb.instructions = out
    return nc


def _emit(ctx, nc, tc, t_in, t_out):
    x_d = t_in["x"]

    singles = ctx.enter_context(tc.tile_pool(name="singles", bufs=1))
    pre_ctx = ExitStack()
    psum_pre = pre_ctx.enter_context(tc.tile_pool(name="psum_pre", bufs=4, space="PSUM"))

    # Wh/Wo first: the Whw precompute sits at the head of the PE queue and must
    # not head-block the x transposes behind a late weight DMA.
    wh_sb = singles.tile([128, 2, C], FP)
    wo_sb = singles.tile([128, 2, C], FPR)
    for kc in range(2):
        nc.sync.dma_start(out=wh_sb[:, kc, :], in_=t_in["Wh"][kc * 128:(kc + 1) * 128, :])
        nc.sync.dma_start(out=wo_sb[:, kc, :], in_=t_in["Wo"][kc * 128:(kc + 1) * 128, :].bitcast(FPR))

    # x, split into 8 chunks so the transposes can start on chunk 0
    x_view = x_d.ap().rearrange("(t p) c -> p t c", p=128)
    x_pix = []
    for q in range(8):
        xp = singles.tile([128, 4, C], FP, name=f"x_pix{q}")
        if q == 0:  # split the first chunk so the first transpose starts sooner
            nc.sync.dma_start(out=xp[:, 0:2, :], in_=x_view[:, 0:2, :])
            nc.sync.dma_start(out=xp[:, 2:4, :], in_=x_view[:, 2:4, :])
        else:
            nc.sync.dma_start(out=xp[:], in_=x_view[:, 4 * q:4 * (q + 1), :])
        x_pix.append(xp)

    identity = singles.tile([128, 128], FP)
    make_identity(nc, identity[:])

    # remaining weights / biases / gamma
    wf_sb = singles.tile([128, 2, CK], FPR)
    wg_sb = singles.tile([128, 2, CK], FPR)
    for kc in range(2):
        nc.sync.dma_start(out=wf_sb[:, kc, :], in_=t_in["Wf"][kc * 128:(kc + 1) * 128, :].bitcast(FPR))
        nc.sync.dma_start(out=wg_sb[:, kc, :], in_=t_in["Wg"][kc * 128:(kc + 1) * 128, :].bitcast(FPR))

    bf_rep = singles.tile([128, 1], FP)
    bg_rep = singles.tile([128, 1], FP)
    for t in range(4):
        nc.sync.dma_start(out=bf_rep[32 * t:32 * t + 32, :], in_=t_in["bf"][:].unsqueeze(1))
        nc.sync.dma_start(out=bg_rep[32 * t:32 * t + 32, :], in_=t_in["bg"][:].unsqueeze(1))

    bh_col = singles.tile([128, 2, 1], FPR)
    bo_row = singles.tile([1, C], FPR)
    for kc in range(2):
        nc.sync.dma_start(
            out=bh_col[:, kc, :], in_=t_in["bh"][kc * 128:(kc + 1) * 128].unsqueeze(1).bitcast(FPR)
        )
    nc.sync.dma_start(out=bo_row[0:1, :], in_=t_in["bo"][:].unsqueeze(0).bitcast(FPR))

    ones_f = singles.tile([1, 128], FP)
    nc.vector.memset(ones_f[:], 1.0)
    ones_col = singles.tile([1, 128], FPR)
    nc.vector.tensor_copy(out=ones_col[:], in_=ones_f[:])
    ones16 = singles.tile([128, 1], BF)
    nc.vector.memset(ones16[:], 1.0)
    ones8 = singles.tile([128, 2, 1], F8)
    nc.vector.memset(ones8[:], 1.0)

    gamma_rep = singles.tile([128, 1], FP)
    nc.sync.dma_start(out=gamma_rep[0:1, :], in_=t_in["gamma"][:].unsqueeze(0))
    for step in range(7):  # doubling broadcast 1 -> 128 partitions
        w_ = 1 << step
        nc.sync.dma_start(out=gamma_rep[w_:2 * w_, :], in_=gamma_rep[0:w_, :])

    # ---------------- fused output projection: Whw = Wh @ Wo, bhw = bh@Wo + bo
    whT = singles.tile([128, 2, C], FPR)  # [m % 128, m // 128, i] = Wh[i, m]
    for mc in range(2):
        for ib in range(2):
            pt = psum_pre.tile([128, 128], FP, tag="transp", bufs=3, name=f"ptw_{mc}_{ib}")
            nc.tensor.transpose(
                pt[:], wh_sb[:, ib, 128 * mc:128 * (mc + 1)], identity[:]
            )
            nc.vector.tensor_copy(out=whT[:, mc, 128 * ib:128 * (ib + 1)], in_=pt[:])

    whw_sb = singles.tile([128, 2, C], FPR)  # [i % 128, i // 128, o]
    for ib in range(2):
        ps = psum_pre.tile([128, C], FP, tag="pre", bufs=2, name=f"psw{ib}")
        for mc in range(2):
            nc.tensor.matmul(
                ps[:],
                whT[:, mc, 128 * ib:128 * (ib + 1)],
                wo_sb[:, mc, :],
                start=(mc == 0),
                stop=(mc == 1),
            )
        nc.vector.tensor_copy(out=whw_sb[:, ib, :], in_=ps[:])

    bhw_bc = singles.tile([128, C], FPR)  # (bh @ Wo + bo) broadcast to all parts
    ps_b = psum_pre.tile([1, C], FP, tag="pre", bufs=2)
    for kc in range(2):
        nc.tensor.matmul(
            ps_b[:], bh_col[:, kc, :], wo_sb[:, kc, :], start=(kc == 0), stop=False
        )
    nc.tensor.matmul(
        ps_b[:], ones_col[0:1, 0:1], bo_row[:], start=False, stop=True
    )
    nc.vector.tensor_copy(out=bhw_bc[0:1, :], in_=ps_b[:])
    for step in range(7):
        w_ = 1 << step
        nc.sync.dma_start(out=bhw_bc[w_:2 * w_, :], in_=bhw_bc[0:w_, :])

    # ---- x^T transposes pipelined with f/g projections, per 512-pixel slice
    # row group t owns keys [1024*t, 1024*(t+1)); fT4[32t:32t+32, j*128:(j+1)*128]
    # holds f^T for global key block 8t+j. gT4 replicates g^T into all 4 groups.
    xtp = pre_ctx.enter_context(tc.tile_pool(name="xtp", bufs=1))
    xT = [[xtp.tile([128, 512], FPR, name=f"xT_{kc}_{s}") for s in range(8)] for kc in range(2)]
    fT4 = singles.tile([128, 1024], FPR)
    gT4 = singles.tile([128, N], FPR)
    fT_flat = singles.tile([32, N], FPR)
    # hw1 = x @ Whw + bhw, split hi/lo fp8, pixel-major [128, t, g4, C] with
    # key block kb = 8t + g4 (attention iteration g4 uses pairs (g4, 8+g4) and
    # (16+g4, 24+g4), i.e. fixed g4, t strided - a regular stride-8C AP).
    hw1hi = singles.tile([128, 4, 8, C], F8)
    hw1lo = singles.tile([128, 4, 8, C], F8)

    for s in range(8):
        for kb in range(4 * s, 4 * s + 4):
            for kc in range(2):
                pt = psum_pre.tile([128, 128], FP, tag="transp", name=f"pt_{kb}_{kc}", bufs=3)
                nc.tensor.transpose(
                    pt[:], x_pix[kb // 4][:, kb % 4, 128 * kc:128 * (kc + 1)], identity[:]
                )
                dst = xT[kc][s][:, 128 * (kb % 4):128 * (kb % 4 + 1)]
                if kc == 0:
                    nc.vector.tensor_copy(out=dst, in_=pt[:])
                else:
                    nc.scalar.activation(out=dst, in_=pt[:], func=AF.Identity, bias=0.0)
        psf = psum_pre.tile([32, 512], FP, tag="pre", name=f"psf{s}", bufs=2)
        for kc in range(2):
            nc.tensor.matmul(
                psf[:],
                wf_sb[:, kc, :],
                xT[kc][s][:],
                start=(kc == 0),
                stop=(kc == 1),
            )
        nc.scalar.activation(
            out=fT_flat[0:32, 512 * s:512 * (s + 1)],
            in_=psf[:],
            func=AF.Identity,
            bias=bf_rep[0:32, :],
        )
        psg = psum_pre.tile([32, 512], FP, tag="pre", name=f"psg{s}", bufs=2)
        for kc in range(2):
            nc.tensor.matmul(
                psg[:],
                wg_sb[:, kc, :],
                xT[kc][s][:],
                start=(kc == 0),
                stop=(kc == 1),
            )
        nc.scalar.activation(
            out=gT4[0:32, 512 * s:512 * (s + 1)],
            in_=psg[:],
            func=AF.Identity,
            bias=bg_rep[0:32, :],
        )
        # hw1 for this slice's pixel blocks: x @ Whw + bhw -> fp8 hi/lo
        for kb in range(4 * s, 4 * s + 4):
            ps = psum_pre.tile([128, C], FP, tag="hw1", name=f"psh{kb}", bufs=2)
            for kc in range(2):
                nc.tensor.matmul(
                    ps[:],
                    xT[kc][kb // 4][:, 128 * (kb % 4):128 * (kb % 4 + 1)],
                    whw_sb[:, kc, :],
                    start=(kc == 0),
                    stop=False,
                )
            nc.tensor.matmul(
                ps[:], ones_col[0:1, :], bhw_bc[0:1, :],
                start=False, stop=True,
            )
            nc.gpsimd.tensor_copy(out=hw1hi[:, kb // 8, kb % 8, :], in_=ps[:])
            nc.vector.tensor_sub(out=hw1lo[:, kb // 8, kb % 8, :], in0=ps[:], in1=hw1hi[:, kb // 8, kb % 8, :])

    for t in range(4):
        nc.sync.dma_start(out=fT4[32 * t:32 * t + 32, :], in_=fT_flat[0:32, 1024 * t:1024 * (t + 1)])
    for t in range(1, 4):
        nc.sync.dma_start(out=gT4[32 * t:32 * t + 32, :], in_=gT4[0:32, :])

    pre_ctx.close()
    et16p = ctx.enter_context(tc.tile_pool(name="et16p", bufs=37))
    et8p = ctx.enter_context(tc.tile_pool(name="et8p", bufs=24))
    work = ctx.enter_context(tc.tile_pool(name="work", bufs=6))
    # PSUM budget (8 banks):
    #   psum_sc : 2 x [128, 1024] score tiles (4 banks)
    #   psum_o  : 2 x [128, 512]  o-accumulators, 2 query chunks per bank
    #   psum_s  : 1 x [128, 4]    row sums of et16 (one bank)
    #   psum_d  : 1 x [128, 4]    row sums of et8 / den (one bank)
    psum_sc = ctx.enter_context(tc.tile_pool(name="psum_sc", bufs=2, space="PSUM"))
    psum_o = ctx.enter_context(tc.tile_pool(name="psum_o", bufs=2, space="PSUM"))
    psum_s = ctx.enter_context(tc.tile_pool(name="psum_s", bufs=1, space="PSUM"))
    psum_d = ctx.enter_context(tc.tile_pool(name="psum_d", bufs=1, space="PSUM"))

    # ---------------- attention --------------------------------------------
    # Software pipeline over 512-query slices:
    #   step qs: scores+exp(qs) | epilogue(qs-2) | pass-2 + value matmul (qs-1)
    #            | row sums(qs) + 1/S broadcast
    # so no engine head-blocks on results produced later in the same step.
    # The last slice (qs=7) instead consumes et16 directly with bf16 value
    # matmuls (normalized by the et16 row sums), so the pipeline drain is just
    # its epilogue instead of a full exposed rescale+matmul tail.
    state = {}

    def phase_scores(qs):
        et16_tiles = []
        for g4 in range(8):
            for pair in range(2):  # pair A: t=0,1 (kb=g4, 8+g4); pair B: t=2,3
                sc = psum_sc.tile([128, 1024], FP, tag="score", name=f"sc_{qs}_{g4}_{pair}")
                for th in range(2):
                    t = 2 * pair + th
                    nc.tensor.matmul(
                        sc[:, 512 * th:512 * (th + 1)],
                        fT4[32 * t:32 * t + 32, 128 * g4:128 * (g4 + 1)],
                        gT4[32 * t:32 * t + 32, 512 * qs:512 * (qs + 1)],
                        start=True,
                        stop=True,
                        tile_position=(32 * t, 0),
                    )
                e16 = et16p.tile([128, 2, 512], BF, tag="et16", name=f"et16_{qs}_{g4}_{pair}")
                nc.scalar.activation(out=e16[:], in_=sc[:], func=AF.Exp)
                et16_tiles.append((g4, pair, e16))
        return et16_tiles

    def phase_sums(qs, et16_tiles):
        # row sums of et16 ride the PE: moving-dim-1 matmuls
        sum_ps = psum_s.tile([128, 4], FP, tag="sum", name=f"sum_{qs}")
        n = len(et16_tiles)
        for idx, (g4, pair, e16) in enumerate(et16_tiles):
            for th in range(2):
                for j in range(4):
                    nc.tensor.matmul(
                        sum_ps[:, j:j + 1],
                        e16[:, th, 128 * j:128 * (j + 1)],
                        ones16[:],
                        start=(idx == 0 and th == 0 and j == 0),
                        stop=(idx == n - 1 and th == 1 and j == 3),
                        skip_group_check=True,
                    )
        # 1/S_q -> bf16 row (gpsimd casting DMAs), then partition broadcast
        rr4 = work.tile([128, 4], FP, tag="rr4", name=f"rr4_{qs}", bufs=2)
        nc.vector.reciprocal(out=rr4[:], in_=sum_ps[:])
        rrow_f = work.tile([1, 512], FP, tag="rrowf", name=f"rrowf_{qs}", bufs=2)
        for j in range(4):
            nc.sync.dma_start(out=rrow_f[0:1, 128 * j:128 * (j + 1)], in_=rr4[:, j:j + 1])
        rrow = work.tile([1, 512], BF, tag="rrow", name=f"rrow_{qs}", bufs=2)
        nc.vector.tensor_copy(out=rrow[:], in_=rrow_f[:])
        rrep = work.tile([128, 2, 512], BF, tag="rrep", name=f"rrep_{qs}", bufs=2)
        for h in range(2):
            nc.gpsimd.partition_broadcast(rrep[:, h, :], rrow[0:1, :])
        return rrep

    def phase_tail(qs, et16_tiles, rrep):
        o_ps = [psum_o.tile([128, 512], FP, tag="oacc", name=f"oacc_{qs}_{c}") for c in range(2)]
        den_ps = psum_d.tile([128, 4], FP, tag="den", name=f"den_{qs}")
        for idx, (g4, pair, e16) in enumerate(et16_tiles):
            e8 = et8p.tile([128, 2, 512], F8, tag="et8", name=f"et8_{qs}_{g4}_{pair}")
            eng = nc.vector if idx % 4 < 3 else nc.gpsimd
            eng.tensor_mul(out=e8[:], in0=e16[:], in1=rrep[:])
            first = g4 == 0 and pair == 0
            last = g4 == 7 and pair == 1
            for j in range(4):
                oc = o_ps[j // 2][:, 256 * (j % 2):256 * (j % 2) + 256]
                lhs = e8[:, :, 128 * j:128 * (j + 1)]
                rhs_hi = hw1hi[:, 2 * pair:2 * pair + 2, g4, :]
                rhs_lo = hw1lo[:, 2 * pair:2 * pair + 2, g4, :]
                nc.tensor.matmul(
                    oc, lhs, rhs_hi,
                    start=(first and j % 2 == 0), stop=False,
                    perf_mode=DR, skip_group_check=True,
                )
                nc.tensor.matmul(
                    oc, lhs, rhs_lo,
                    start=False, stop=(last and j % 2 == 1),
                    perf_mode=DR, skip_group_check=True,
                )
                nc.tensor.matmul(
                    den_ps[:, j:j + 1], lhs, ones8[:],
                    start=(first and j == 0), stop=(last and j == 3),
                    perf_mode=DR, skip_group_check=True,
                )
        return o_ps, den_ps

    def phase_tail_bf16(qs, et16_tiles):
        # last slice: bf16 value matmul straight from et16 (no rescale barrier);
        # normalized by the et16 row sums.
        o_ps = [psum_o.tile([128, 512], FP, tag="oacc", name=f"oacc_{qs}_{c}") for c in range(2)]
        for idx, (g4, pair, e16) in enumerate(et16_tiles):
            first = g4 == 0 and pair == 0
            last = g4 == 7 and pair == 1
            for j in range(4):
                oc = o_ps[j // 2][:, 256 * (j % 2):256 * (j % 2) + 256]
                for th in range(2):
                    t = 2 * pair + th
                    nc.tensor.matmul(
                        oc,
                        e16[:, th, 128 * j:128 * (j + 1)],
                        hw1bf[:, t, g4, :],
                        start=(first and th == 0 and j % 2 == 0),
                        stop=(last and th == 1 and j % 2 == 1),
                        skip_group_check=True,
                    )
        return o_ps

    def phase_epilogue(qs, o_ps, den_ps):
        # out = gamma * (num / den) + x
        dinv = work.tile([128, 4], FP, tag="dinv", name=f"dinv_{qs}", bufs=2)
        nc.vector.reciprocal(out=dinv[:], in_=den_ps[:])
        nc.vector.tensor_scalar(
            out=dinv[:], in0=dinv[:], scalar1=gamma_rep[:], scalar2=None, op0=ALU.mult
        )
        for j in range(4):
            blk = 4 * qs + j
            out_sb = work.tile([128, C], FP, tag="outsb", name=f"osb_{blk}", bufs=3)
            nc.vector.scalar_tensor_tensor(
                out=out_sb[:],
                in0=o_ps[j // 2][:, 256 * (j % 2):256 * (j % 2) + 256],
                scalar=dinv[:, j:j + 1],
                in1=x_pix[blk // 4][:, blk % 4, :],
                op0=ALU.mult,
                op1=ALU.add,
            )
            nc.sync.dma_start(out=t_out[128 * blk:128 * (blk + 1), :], in_=out_sb[:])

    for step in range(11):
        if step >= 3:  # epilogue(step-3) first: frees o-banks / den for this step
            st = state.pop(step - 3)
            phase_epilogue(step - 3, st["o"], st["den"])
        if step < 8:
            state[step] = {"et16": phase_scores(step)}
        if 1 <= step <= 8:
            state[step - 1]["rrep"] = phase_sums(step - 1, state[step - 1]["et16"])
        if 2 <= step <= 9:
            st = state[step - 2]
            st["o"], st["den"] = phase_tail(step - 2, st["et16"], st["rrep"])


_CACHE = {}


def _build():
    if "nc" not in _CACHE:
        nc = bass.Bass("TRN2", target_bir_lowering=False, debug=False)
        t_in = {
            "x": nc.dram_tensor("x", [N, C], FP, kind="ExternalInput"),
            "Wf": nc.dram_tensor("Wf", [C, CK], FP, kind="ExternalInput"),
            "bf": nc.dram_tensor("bf", [CK], FP, kind="ExternalInput"),
            "Wg": nc.dram_tensor("Wg", [C, CK], FP, kind="ExternalInput"),
            "bg": nc.dram_tensor("bg", [CK], FP, kind="ExternalInput"),
            "Wh": nc.dram_tensor("Wh", [C, C], FP, kind="ExternalInput"),
            "bh": nc.dram_tensor("bh", [C], FP, kind="ExternalInput"),
            "Wo": nc.dram_tensor("Wo", [C, C], FP, kind="ExternalInput"),
            "bo": nc.dram_tensor("bo", [C], FP, kind="ExternalInput"),
            "gamma": nc.dram_tensor("gamma", [1], FP, kind="ExternalInput"),
        }
        t_out = nc.dram_tensor("out", [N, C], FP, kind="ExternalOutput")
        with tile.TileContext(nc) as tc:
            with ExitStack() as ctx:
                _emit(ctx, nc, tc, t_in, t_out)
        _split_instruction_waits(nc)
        _CACHE["nc"] = nc
    return _CACHE["nc"]


def kernel(x, Wf, bf, Wg, bg, Wh, bh, Wo, bo, gamma, _trace=False, _tmpdir=None):
    nc = _build()
    x = np.ascontiguousarray(np.asarray(x, dtype=np.float32)).reshape(B, N, C)
    w = {
        "Wf": np.ascontiguousarray(np.asarray(Wf, np.float32)),
        "bf": np.ascontiguousarray(np.asarray(bf, np.float32)),
        "Wg": np.ascontiguousarray(np.asarray(Wg, np.float32)),
        "bg": np.ascontiguousarray(np.asarray(bg, np.float32)),
        "Wh": np.ascontiguousarray(np.asarray(Wh, np.float32)),
        "bh": np.ascontiguousarray(np.asarray(bh, np.float32)),
        "Wo": np.ascontiguousarray(np.asarray(Wo, np.float32)),
        "bo": np.ascontiguousarray(np.asarray(bo, np.float32)),
        "gamma": np.ascontiguousarray(np.asarray(gamma, np.float32)),
    }
    in_maps = [dict(w, x=x[i]) for i in range(NCORES)]
    res = run_bass_kernel_spmd(
        nc, in_maps, core_ids=list(range(NCORES)), trace=_trace, tmpdir=_tmpdir
    )
    out = np.stack([res.results[i]["out"] for i in range(NCORES)])
    if _trace:
        kernel._last_result = res
    return out.reshape(B, H, W, C).astype(np.float32)
